# revision 10
# baseline (speedup 1.0000x reference)
"""Trainium2 Bass kernel for nn_BaselineDecoder4 (2-layer LSTM decoder +
masked-mean "attention" + vocab projection), V-sharded across 8 cores.

Key algebraic facts exploited:
 - The reference softmax over S is applied to scores equal to att[t,b,h]
   on every unmasked s, so the attention weights collapse to
   1/n_unmasked[b] on unmasked positions -> ctx[b,h] is the masked mean
   of enc_output over S, independent of t and of W1/dense1 entirely.
 - out = [dec, ctx] @ W2.T = dec @ W2a.T + (ctx @ W2b.T); the second term
   is t-invariant -> computed once as Bc[b, v], added via an extra K=32
   broadcast matmul into the logits PSUM group.
All matmuls run fp16 (fp32 PSUM accumulation); cell state kept fp32.
Layout: gates/hidden "tile layout" [p, j*32+b] for unit u = 128*j + p,
with gate blocks permuted to (i, f, o, g) so one sigmoid covers i,f,o.
"""
import sys
sys.path.insert(0, '/opt/trn_rl_repo')
import numpy as np
from contextlib import ExitStack

from concourse import bacc, bass, mybir, tile
from concourse.bass_utils import run_bass_kernel_spmd
from concourse.masks import make_identity

dt = mybir.dt
AF = mybir.ActivationFunctionType
ALU = mybir.AluOpType

V, H, L, T, S, B = 32000, 512, 2, 64, 64, 32
NC = 8
VS = V // NC          # 4000 vocab rows per core
TB = T * B            # 2048
NG = 16               # 128-row tiles covering TB / SB
BLK = 16              # steps per block
LAG = 16
F16 = dt.float16
F32 = dt.float32

VSP = 4096            # padded vocab shard
NVC = VSP // 512      # 8 v-chunks per core


def build_nc():
    nc = bacc.Bacc(None)
    d = {}
    d["tgt"] = nc.declare_dram_parameter("tgt", [TB, 1], dt.int32, isOutput=False)
    d["emb"] = nc.declare_dram_parameter("emb", [V, H], F32, isOutput=False)
    d["enc"] = nc.declare_dram_parameter("enc", [S * B, H], F32, isOutput=False)
    d["msk"] = nc.declare_dram_parameter("msk", [S * B, 1], dt.int32, isOutput=False)
    d["h0"] = nc.declare_dram_parameter("h0", [L, B, H], F32, isOutput=False)
    d["c0"] = nc.declare_dram_parameter("c0", [L, B, H], F32, isOutput=False)
    for l in range(L):
        d[f"wih{l}"] = nc.declare_dram_parameter(f"wih{l}", [128, 4, 4 * H], F16, isOutput=False)
        d[f"whh{l}"] = nc.declare_dram_parameter(f"whh{l}", [128, 4, 4 * H], F16, isOutput=False)
        d[f"bias{l}"] = nc.declare_dram_parameter(f"bias{l}", [128, 16], F32, isOutput=False)
    d["w2a"] = nc.declare_dram_parameter("w2a", [NVC, 128, 4, 512], F16, isOutput=False)
    d["w2b"] = nc.declare_dram_parameter("w2b", [NVC, 128, 4, 512], F16, isOutput=False)
    d["out"] = nc.declare_dram_parameter("out", [NG, NVC, 128, 512], F32, isOutput=True)
    d["hso"] = nc.declare_dram_parameter("hso", [L, 128, 4, 32], F32, isOutput=True)
    d["cso"] = nc.declare_dram_parameter("cso", [L, 128, 4, 32], F32, isOutput=True)

    with tile.TileContext(nc) as tc, ExitStack() as ctx:
        const = ctx.enter_context(tc.tile_pool(name="const", bufs=1))
        wsb = ctx.enter_context(tc.tile_pool(name="wsb", bufs=1))
        big = ctx.enter_context(tc.tile_pool(name="big", bufs=1))
        stage = ctx.enter_context(tc.tile_pool(name="stage", bufs=2))
        ring = ctx.enter_context(tc.tile_pool(name="ring", bufs=1))
        chain = ctx.enter_context(tc.tile_pool(name="chain", bufs=2))
        outp = ctx.enter_context(tc.tile_pool(name="outp", bufs=2))
        pg = ctx.enter_context(tc.tile_pool(name="pg", bufs=2, space="PSUM"))
        pw = ctx.enter_context(tc.tile_pool(name="pw", bufs=2, space="PSUM"))
        pt = ctx.enter_context(tc.tile_pool(name="pt", bufs=2, space="PSUM"))
        psm = ctx.enter_context(tc.tile_pool(name="psm", bufs=2, space="PSUM"))

        # ---------------- constants / small loads ----------------
        ident = const.tile([128, 128], F32)
        make_identity(nc, ident)

        idx_t = const.tile([128, NG], dt.int32, tag="idx")
        nc.sync.dma_start(out=idx_t, in_=d["tgt"].rearrange("(c p) o -> p (c o)", p=128))

        bias_sb = []
        for l in range(L):
            bt = const.tile([128, 16], F32, tag=f"bias{l}")
            nc.sync.dma_start(out=bt, in_=d[f"bias{l}"][:, :])
            bias_sb.append(bt)

        # LSTM weights resident: [128, hc, 4H] fp16 (rows = h within chunk hc)
        whh, wih = [], []
        for l in range(L):
            wt = wsb.tile([128, 4, 4 * H], F16, tag=f"whh{l}")
            nc.sync.dma_start(out=wt, in_=d[f"whh{l}"][:, :, :])
            whh.append(wt)
            wt = wsb.tile([128, 4, 4 * H], F16, tag=f"wih{l}")
            nc.sync.dma_start(out=wt, in_=d[f"wih{l}"][:, :, :])
            wih.append(wt)

        # mask -> onem = (1 - m) [128, 16] fp32 + fp16
        msk_t = const.tile([128, NG], dt.int32, tag="msk")
        nc.sync.dma_start(out=msk_t, in_=d["msk"].rearrange("(c p) o -> p (c o)", p=128))
        onem = const.tile([128, NG], F32, tag="onem")
        nc.vector.tensor_scalar(onem, msk_t, -1.0, 1.0, op0=ALU.mult, op1=ALU.add)
        onem16 = const.tile([128, NG], F16, tag="onem16")
        nc.vector.tensor_copy(onem16, onem)

        # base selection matrix [128, 32]: (p % 32 == b)
        colj = const.tile([128, 32], dt.int32, tag="colj")
        nc.gpsimd.iota(colj, pattern=[[1, 32]], base=0, channel_multiplier=0)
        prow = const.tile([128, 32], dt.int32, tag="prow")
        nc.gpsimd.iota(prow, pattern=[[0, 32]], base=0, channel_multiplier=1)
        diff = const.tile([128, 32], dt.int32, tag="diff")
        nc.vector.tensor_tensor(diff, prow, colj, op=ALU.subtract)
        base_f = const.tile([128, 32], F32, tag="base_f")
        nc.vector.memset(base_f, 0.0)
        for v0 in (0, 32, 64, 96):
            eq1 = stage.tile([128, 32], F32, tag="eq1")
            nc.vector.tensor_scalar(eq1, diff, v0, None, op0=ALU.is_equal)
            nc.vector.tensor_tensor(base_f, base_f, eq1, op=ALU.add)
        base16 = const.tile([128, 32], F16, tag="base16")
        nc.vector.tensor_copy(base16, base_f)
        # transposed selection [32, 128] fp16 (for Bc broadcast matmul)
        pst0 = pt.tile([128, 128], F32, tag="ptr")
        nc.tensor.transpose(out=pst0[0:32, 0:128], in_=base_f, identity=ident)
        base16T = const.tile([32, 128], F16, tag="base16T")
        nc.vector.tensor_copy(base16T, pst0[0:32, 0:128])

        # ---------------- embedding gather + transpose -> xT ----------------
        xT = big.tile([128, 4, TB], F16, tag="xT")  # [h%128, hc, tb]
        for g in range(NG):
            xg = stage.tile([128, H], F32, tag="ldf32")
            nc.gpsimd.indirect_dma_start(
                out=xg, out_offset=None, in_=d["emb"][:, :],
                in_offset=bass.IndirectOffsetOnAxis(ap=idx_t[:, g:g + 1], axis=0))
            for hc in range(4):
                pst = pt.tile([128, 128], F32, tag="ptr")
                nc.tensor.transpose(out=pst, in_=xg[:, hc * 128:(hc + 1) * 128],
                                    identity=ident)
                nc.vector.tensor_copy(xT[:, hc, g * 128:(g + 1) * 128], pst)

        # ---------------- ctx: masked mean of enc over S ----------------
        ps_ctx = psm.tile([32, 512], F32, tag="psmall")
        ps_n = psm.tile([32, 512], F32, tag="psmall")
        for g in range(NG):
            eg = stage.tile([128, H], F32, tag="ldf32")
            nc.sync.dma_start(out=eg, in_=d["enc"][g * 128:(g + 1) * 128, :])
            eg16 = stage.tile([128, H], F16, tag="ld16")
            nc.vector.tensor_copy(eg16, eg)
            msel = stage.tile([128, 32], F16, tag="msel")
            nc.vector.tensor_scalar(msel, base16, onem[:, g:g + 1], None, op0=ALU.mult)
            nc.tensor.matmul(ps_ctx, msel, eg16, start=(g == 0), stop=(g == NG - 1),
                             skip_group_check=True)
            nc.tensor.matmul(ps_n[:, 0:1], base16, onem16[:, g:g + 1],
                             start=(g == 0), stop=(g == NG - 1), skip_group_check=True)
        n_sb = const.tile([32, 1], F32, tag="n_sb")
        nc.vector.tensor_copy(n_sb, ps_n[:, 0:1])
        inv_n = const.tile([32, 1], F32, tag="inv_n")
        nc.vector.reciprocal(inv_n, n_sb)
        ctx_sb = const.tile([32, H], F32, tag="ctx_sb")
        nc.scalar.activation(out=ctx_sb, in_=ps_ctx, func=AF.Copy, scale=inv_n[:, 0:1])
        ctxT = const.tile([128, 4, 32], F16, tag="ctxT")
        for hc in range(4):
            pst = pt.tile([128, 128], F32, tag="ptr")
            nc.tensor.transpose(out=pst[:, 0:32], in_=ctx_sb[:, hc * 128:(hc + 1) * 128],
                                identity=ident[0:32, 0:32])
            nc.vector.tensor_copy(ctxT[:, hc, :], pst[:, 0:32])

        # Bc[b, v] = ctx @ W2b : [32, VS] fp16
        bc16 = const.tile([32, VSP], F16, tag="bc16")
        for vi in range(NVC):
            wbt = outp.tile([128, 4, 512], F16, tag="w2t")
            nc.scalar.dma_start(out=wbt, in_=d["w2b"][vi])
            ps_bc = psm.tile([32, 512], F32, tag="psmall")
            for hc in range(4):
                nc.tensor.matmul(ps_bc, ctxT[:, hc, :], wbt[:, hc, :],
                                 start=(hc == 0), stop=(hc == 3))
            nc.vector.tensor_copy(bc16[:, vi * 512:(vi + 1) * 512], ps_bc)

        # ---------------- h0/c0 -> tile layout ----------------
        h_cur, c_cur = [], []
        for l in range(L):
            hs_in = stage.tile([32, H], F32, tag="ldf32")
            nc.sync.dma_start(out=hs_in, in_=d["h0"][l, :, :])
            cs_in = stage.tile([32, H], F32, tag="ldf32")
            nc.sync.dma_start(out=cs_in, in_=d["c0"][l, :, :])
            ht = chain.tile([128, 128], F16, tag=f"h{l}")
            ct = chain.tile([128, 128], F32, tag=f"c{l}")
            for hc in range(4):
                pst = pt.tile([128, 128], F32, tag="ptr")
                nc.tensor.transpose(out=pst[:, 0:32], in_=hs_in[:, hc * 128:(hc + 1) * 128],
                                    identity=ident[0:32, 0:32])
                nc.vector.tensor_copy(ht[:, hc * 32:(hc + 1) * 32], pst[:, 0:32])
                pst2 = pt.tile([128, 128], F32, tag="ptr")
                nc.tensor.transpose(out=pst2[:, 0:32], in_=cs_in[:, hc * 128:(hc + 1) * 128],
                                    identity=ident[0:32, 0:32])
                nc.vector.tensor_copy(ct[:, hc * 32:(hc + 1) * 32], pst2[:, 0:32])
            h_cur.append(ht)
            c_cur.append(ct)

        # ---------------- persistent big tiles ----------------
        h1T = big.tile([128, 4, TB], F16, tag="h1T")    # layer-1 outputs
        decT = big.tile([128, 4, TB], F16, tag="decT")  # layer-2 outputs

        def emit_xw_block(l, k, rhs_big):
            """xw (input projection + bias) for steps [16k, 16k+16) of
            layer l from rhs_big [128, hc, TB] fp16."""
            xwt = ring.tile([128, BLK, 512], F16, tag=f"xwr{l}")
            for gc in range(16):
                ps_x = pw.tile([128, 512], F32, tag="pwork")
                for hc in range(4):
                    nc.tensor.matmul(
                        ps_x, wih[l][:, hc, gc * 128:(gc + 1) * 128],
                        rhs_big[:, hc, k * 512:(k + 1) * 512],
                        start=(hc == 0), stop=(hc == 3))
                nc.scalar.activation(
                    out=xwt[:, :, gc * 32:(gc + 1) * 32],
                    in_=ps_x.rearrange("p (t b) -> p t b", b=32),
                    func=AF.Identity, bias=bias_sb[l][:, gc:gc + 1])
            return xwt

        def emit_step(l, t, xwt, final):
            """One LSTM step: 64 rec MMs + gate chain; updates h/c state."""
            ht_prev, ct_prev = h_cur[l], c_cur[l]
            ps_g = pg.tile([128, 512], F32, tag="pgate")
            for gc in range(16):
                for hc in range(4):
                    nc.tensor.matmul(
                        ps_g[:, gc * 32:(gc + 1) * 32],
                        whh[l][:, hc, gc * 128:(gc + 1) * 128],
                        ht_prev[:, hc * 32:(hc + 1) * 32],
                        start=(hc == 0), stop=(hc == 3), skip_group_check=True)
            s_sb = chain.tile([128, 512], F32, tag="s_sb")
            nc.vector.tensor_tensor(s_sb, ps_g, xwt[:, t % BLK, :], op=ALU.add)
            sif = chain.tile([128, 384], F32, tag="sif")
            nc.scalar.activation(out=sif, in_=s_sb[:, 0:384], func=AF.Sigmoid)
            tg = chain.tile([128, 128], F32, tag="tg")
            nc.scalar.activation(out=tg, in_=s_sb[:, 384:512], func=AF.Tanh)
            ve = nc.gpsimd if l == 0 else nc.vector
            t1 = chain.tile([128, 128], F32, tag="t1")
            ve.tensor_mul(t1, sif[:, 0:128], tg)
            t2 = chain.tile([128, 128], F32, tag="t2")
            ve.tensor_mul(t2, sif[:, 128:256], ct_prev)
            ct_new = chain.tile([128, 128], F32, tag=f"c{l}")
            ve.tensor_add(ct_new, t1, t2)
            tcn = chain.tile([128, 128], F32, tag="tcn")
            nc.scalar.activation(out=tcn, in_=ct_new, func=AF.Tanh)
            ht_new = chain.tile([128, 128], F16, tag=f"h{l}")
            nc.vector.tensor_mul(ht_new, sif[:, 256:384], tcn)
            h_cur[l], c_cur[l] = ht_new, ct_new
            dst = h1T if l == 0 else decT
            for hc in range(4):
                nc.vector.tensor_copy(dst[:, hc, t * 32:(t + 1) * 32],
                                      ht_new[:, hc * 32:(hc + 1) * 32])
            if final:
                hf = const.tile([128, 128], F32, tag=f"hf{l}")
                nc.vector.tensor_mul(hf, sif[:, 256:384], tcn)
                nc.sync.dma_start(out=d["hso"][l],
                                  in_=hf.rearrange("p (j b) -> p j b", b=32))
                nc.sync.dma_start(out=d["cso"][l],
                                  in_=ct_new.rearrange("p (j b) -> p j b", b=32))

        logits_sched = []

        def emit_logits_group(tp, vi):
            """Logits for tb-chunk pair tp (tbc 2tp, 2tp+1), v-chunk vi."""
            w2t = outp.tile([128, 4, 512], F16, tag="w2t")
            nc.scalar.dma_start(out=w2t, in_=d["w2a"][vi])
            for tbc in (2 * tp, 2 * tp + 1):
                tb0 = tbc * 128
                ps_l = pw.tile([128, 512], F32, tag="pwork")
                for hc in range(4):
                    nc.tensor.matmul(
                        ps_l, decT[:, hc, tb0:tb0 + 128], w2t[:, hc, :],
                        start=(hc == 0), stop=False)
                nc.tensor.matmul(ps_l, base16T, bc16[:, vi * 512:(vi + 1) * 512],
                                 start=False, stop=True)
                ot = outp.tile([128, 512], F32, tag="ot")
                if (tbc + vi) % 2 == 0:
                    nc.vector.tensor_copy(ot, ps_l)
                    nc.sync.dma_start(out=d["out"][tbc, vi], in_=ot)
                else:
                    nc.scalar.copy(ot, ps_l)
                    nc.scalar.dma_start(out=d["out"][tbc, vi], in_=ot)

        # ---------------- main interleaved schedule ----------------
        xw_cur = [None, None]
        xw_cur[0] = emit_xw_block(0, 0, xT)
        for slot in range(T + LAG):
            t1_ = slot
            t2_ = slot - LAG
            if t1_ < T:
                if t1_ % BLK == 0 and t1_ > 0:
                    xw_cur[0] = emit_xw_block(0, t1_ // BLK, xT)
                emit_step(0, t1_, xw_cur[0], final=(t1_ == T - 1))
            if 0 <= t2_ < T:
                if t2_ % BLK == 0:
                    xw_cur[1] = emit_xw_block(1, t2_ // BLK, h1T)
                emit_step(1, t2_, xw_cur[1], final=(t2_ == T - 1))
                if t2_ % 8 == 7:
                    for vi in range(NVC):
                        logits_sched.append((t2_ // 8, vi))
            for _ in range(2):
                if logits_sched:
                    emit_logits_group(*logits_sched.pop(0))
        while logits_sched:
            emit_logits_group(*logits_sched.pop(0))

    nc.finalize()
    return nc


_NC_CACHE = None


def kernel(**inputs):
    global _NC_CACHE
    tgt = np.asarray(inputs["tgt"]).astype(np.int32).reshape(TB, 1)
    emb = np.asarray(inputs["emb"], dtype=np.float32)
    enc = np.asarray(inputs["enc_output"], dtype=np.float32).reshape(S * B, H)
    msk = np.asarray(inputs["src_mask"]).astype(np.int32).reshape(S * B, 1)
    hid = np.asarray(inputs["hidden"], dtype=np.float32)
    cel = np.asarray(inputs["cell"], dtype=np.float32)
    w2 = np.asarray(inputs["W2"], dtype=np.float32)  # [V, 2H]

    # gate-permuted (i, f, o, g) fp16 weights, transposed to [H, 4H]
    perm = np.concatenate([np.arange(0, 1024), np.arange(1536, 2048),
                           np.arange(1024, 1536)])
    base = {"tgt": tgt, "emb": emb, "enc": enc, "msk": msk, "h0": hid, "c0": cel}
    for l in range(L):
        wihl = np.asarray(inputs[f"W_ih_l{l}"], dtype=np.float32)[perm]
        whhl = np.asarray(inputs[f"W_hh_l{l}"], dtype=np.float32)[perm]
        bl = (np.asarray(inputs[f"b_ih_l{l}"], dtype=np.float32)
              + np.asarray(inputs[f"b_hh_l{l}"], dtype=np.float32))[perm]
        # [H, 4H] -> [128, hc, 4H] (partition-contiguous)
        base[f"wih{l}"] = np.ascontiguousarray(
            wihl.T.reshape(4, 128, 4 * H).transpose(1, 0, 2)).astype(np.float16)
        base[f"whh{l}"] = np.ascontiguousarray(
            whhl.T.reshape(4, 128, 4 * H).transpose(1, 0, 2)).astype(np.float16)
        base[f"bias{l}"] = np.ascontiguousarray(bl.reshape(16, 128).T).astype(np.float32)

    w2a = w2[:, :H].T.astype(np.float16)  # [H, V]
    w2b = w2[:, H:].T.astype(np.float16)

    def shard_w2(w, k):
        # [H, VS] -> pad to [H, VSP] -> [vi, p, hc, vv] chunk-major
        ws = np.zeros((H, VSP), np.float16)
        ws[:, :VS] = w[:, k * VS:(k + 1) * VS]
        return np.ascontiguousarray(
            ws.reshape(4, 128, NVC, 512).transpose(2, 1, 0, 3))

    in_maps = []
    for k in range(NC):
        m = dict(base)
        m["w2a"] = shard_w2(w2a, k)
        m["w2b"] = shard_w2(w2b, k)
        in_maps.append(m)

    if _NC_CACHE is None:
        _NC_CACHE = build_nc()
    res = run_bass_kernel_spmd(_NC_CACHE, in_maps, list(range(NC)))
    outs = np.concatenate(
        [res.results[k]["out"].transpose(0, 2, 1, 3).reshape(TB, VSP)[:, :VS]
         .reshape(T, B, VS) for k in range(NC)], axis=-1)
    # device layout [l, p, j, b] -> [l, b, u=128j+p]
    hs = res.results[0]["hso"].transpose(0, 3, 2, 1).reshape(L, B, H)
    cs = res.results[0]["cso"].transpose(0, 3, 2, 1).reshape(L, B, H)
    return outs, hs, cs


# revision 11
# speedup vs baseline: 1.1088x; 1.1088x over previous
"""Trainium2 Bass kernel for nn_BaselineDecoder4 (2-layer LSTM decoder +
masked-mean "attention" + vocab projection), V-sharded across 8 cores.

Key algebraic facts exploited:
 - The reference softmax over S is applied to scores equal to att[t,b,h]
   on every unmasked s, so the attention weights collapse to
   1/n_unmasked[b] on unmasked positions -> ctx[b,h] is the masked mean
   of enc_output over S, independent of t and of W1/dense1 entirely.
 - out = [dec, ctx] @ W2.T = dec @ W2a.T + (ctx @ W2b.T); the second term
   is t-invariant -> computed once as Bc[b, v], added via an extra K=32
   broadcast matmul into the logits PSUM group.
All matmuls run fp16 (fp32 PSUM accumulation); cell state kept fp32.
Layout: gates/hidden "tile layout" [p, j*32+b] for unit u = 128*j + p,
with gate blocks permuted to (i, f, o, g) so one sigmoid covers i,f,o.
"""
import sys
sys.path.insert(0, '/opt/trn_rl_repo')
import numpy as np
from contextlib import ExitStack

from concourse import bacc, bass, mybir, tile
from concourse.bass_utils import run_bass_kernel_spmd
from concourse.masks import make_identity

dt = mybir.dt
AF = mybir.ActivationFunctionType
ALU = mybir.AluOpType

V, H, L, T, S, B = 32000, 512, 2, 64, 64, 32
NC = 8
VS = V // NC          # 4000 vocab rows per core
TB = T * B            # 2048
NG = 16               # 128-row tiles covering TB / SB
BLK = 16              # steps per block
LAG = 16
F16 = dt.float16
F32 = dt.float32

VSP = 4096            # padded vocab shard
NVC = VSP // 512      # 8 v-chunks per core


def build_nc():
    nc = bacc.Bacc(None)
    d = {}
    d["tgt"] = nc.declare_dram_parameter("tgt", [TB, 1], dt.int32, isOutput=False)
    d["emb"] = nc.declare_dram_parameter("emb", [V, H], F32, isOutput=False)
    d["enc"] = nc.declare_dram_parameter("enc", [S * B, H], F32, isOutput=False)
    d["msk"] = nc.declare_dram_parameter("msk", [S * B, 1], dt.int32, isOutput=False)
    d["h0"] = nc.declare_dram_parameter("h0", [L, B, H], F32, isOutput=False)
    d["c0"] = nc.declare_dram_parameter("c0", [L, B, H], F32, isOutput=False)
    for l in range(L):
        d[f"wih{l}"] = nc.declare_dram_parameter(f"wih{l}", [128, 4, 4 * H], F16, isOutput=False)
        d[f"whh{l}"] = nc.declare_dram_parameter(f"whh{l}", [128, 4, 4 * H], F16, isOutput=False)
        d[f"bias{l}"] = nc.declare_dram_parameter(f"bias{l}", [128, 16], F32, isOutput=False)
    d["w2a"] = nc.declare_dram_parameter("w2a", [NVC, 128, 4, 512], F16, isOutput=False)
    d["w2b"] = nc.declare_dram_parameter("w2b", [NVC, 128, 4, 512], F16, isOutput=False)
    d["out"] = nc.declare_dram_parameter("out", [NG, NVC, 128, 512], F32, isOutput=True)
    d["hso"] = nc.declare_dram_parameter("hso", [L, 128, 4, 32], F32, isOutput=True)
    d["cso"] = nc.declare_dram_parameter("cso", [L, 128, 4, 32], F32, isOutput=True)

    with tile.TileContext(nc) as tc, ExitStack() as ctx:
        const = ctx.enter_context(tc.tile_pool(name="const", bufs=1))
        wsb = ctx.enter_context(tc.tile_pool(name="wsb", bufs=1))
        big = ctx.enter_context(tc.tile_pool(name="big", bufs=1))
        stage = ctx.enter_context(tc.tile_pool(name="stage", bufs=2))
        ring = ctx.enter_context(tc.tile_pool(name="ring", bufs=1))
        chain = ctx.enter_context(tc.tile_pool(name="chain", bufs=2))
        outp = ctx.enter_context(tc.tile_pool(name="outp", bufs=2))
        pg = ctx.enter_context(tc.tile_pool(name="pg", bufs=2, space="PSUM"))
        pw = ctx.enter_context(tc.tile_pool(name="pw", bufs=2, space="PSUM"))
        pt = ctx.enter_context(tc.tile_pool(name="pt", bufs=2, space="PSUM"))
        psm = ctx.enter_context(tc.tile_pool(name="psm", bufs=2, space="PSUM"))

        # ---------------- constants / small loads ----------------
        ident = const.tile([128, 128], F32)
        make_identity(nc, ident)
        ident16 = const.tile([128, 128], F16, tag="ident16")
        nc.vector.tensor_copy(ident16, ident)

        idx_t = const.tile([128, NG], dt.int32, tag="idx")
        nc.sync.dma_start(out=idx_t, in_=d["tgt"].rearrange("(c p) o -> p (c o)", p=128))

        bias_sb = []
        for l in range(L):
            bt = const.tile([128, 16], F32, tag=f"bias{l}")
            nc.sync.dma_start(out=bt, in_=d[f"bias{l}"][:, :])
            bias_sb.append(bt)

        # LSTM weights resident: [128, hc, 4H] fp16 (rows = h within chunk hc)
        whh, wih = [], []
        for l in range(L):
            wt = wsb.tile([128, 4, 4 * H], F16, tag=f"whh{l}")
            nc.sync.dma_start(out=wt, in_=d[f"whh{l}"][:, :, :])
            whh.append(wt)
            wt = wsb.tile([128, 4, 4 * H], F16, tag=f"wih{l}")
            nc.sync.dma_start(out=wt, in_=d[f"wih{l}"][:, :, :])
            wih.append(wt)

        # mask -> onem = (1 - m) [128, 16] fp32 + fp16
        msk_t = const.tile([128, NG], dt.int32, tag="msk")
        nc.sync.dma_start(out=msk_t, in_=d["msk"].rearrange("(c p) o -> p (c o)", p=128))
        onem = const.tile([128, NG], F32, tag="onem")
        nc.vector.tensor_scalar(onem, msk_t, -1.0, 1.0, op0=ALU.mult, op1=ALU.add)
        onem16 = const.tile([128, NG], F16, tag="onem16")
        nc.vector.tensor_copy(onem16, onem)

        # base selection matrix [128, 32]: (p % 32 == b)
        colj = const.tile([128, 32], dt.int32, tag="colj")
        nc.gpsimd.iota(colj, pattern=[[1, 32]], base=0, channel_multiplier=0)
        prow = const.tile([128, 32], dt.int32, tag="prow")
        nc.gpsimd.iota(prow, pattern=[[0, 32]], base=0, channel_multiplier=1)
        diff = const.tile([128, 32], dt.int32, tag="diff")
        nc.vector.tensor_tensor(diff, prow, colj, op=ALU.subtract)
        base_f = const.tile([128, 32], F32, tag="base_f")
        nc.vector.memset(base_f, 0.0)
        for v0 in (0, 32, 64, 96):
            eq1 = stage.tile([128, 32], F32, tag="eq1")
            nc.vector.tensor_scalar(eq1, diff, v0, None, op0=ALU.is_equal)
            nc.vector.tensor_tensor(base_f, base_f, eq1, op=ALU.add)
        base16 = const.tile([128, 32], F16, tag="base16")
        nc.vector.tensor_copy(base16, base_f)
        # transposed selection [32, 128] fp16 (for Bc broadcast matmul)
        pst0 = pt.tile([128, 128], F32, tag="ptr")
        nc.tensor.transpose(out=pst0[0:32, 0:128], in_=base_f, identity=ident)
        base16T = const.tile([32, 128], F16, tag="base16T")
        nc.vector.tensor_copy(base16T, pst0[0:32, 0:128])

        # ---------------- embedding gather + transpose -> xT ----------------
        xT = big.tile([128, 4, TB], F16, tag="xT")  # [h%128, hc, tb]
        for g in range(NG):
            xg = stage.tile([128, H], F32, tag="ldf32")
            nc.gpsimd.indirect_dma_start(
                out=xg, out_offset=None, in_=d["emb"][:, :],
                in_offset=bass.IndirectOffsetOnAxis(ap=idx_t[:, g:g + 1], axis=0))
            for hc in range(4):
                pst = pt.tile([128, 128], F32, tag="ptr")
                nc.tensor.transpose(out=pst, in_=xg[:, hc * 128:(hc + 1) * 128],
                                    identity=ident)
                nc.vector.tensor_copy(xT[:, hc, g * 128:(g + 1) * 128], pst)

        # ---------------- ctx: masked mean of enc over S ----------------
        ps_ctx = psm.tile([32, 512], F32, tag="psmall")
        ps_n = psm.tile([32, 512], F32, tag="psmall")
        for g in range(NG):
            eg = stage.tile([128, H], F32, tag="ldf32")
            nc.sync.dma_start(out=eg, in_=d["enc"][g * 128:(g + 1) * 128, :])
            eg16 = stage.tile([128, H], F16, tag="ld16")
            nc.vector.tensor_copy(eg16, eg)
            msel = stage.tile([128, 32], F16, tag="msel")
            nc.vector.tensor_scalar(msel, base16, onem[:, g:g + 1], None, op0=ALU.mult)
            nc.tensor.matmul(ps_ctx, msel, eg16, start=(g == 0), stop=(g == NG - 1),
                             skip_group_check=True)
            nc.tensor.matmul(ps_n[:, 0:1], base16, onem16[:, g:g + 1],
                             start=(g == 0), stop=(g == NG - 1), skip_group_check=True)
        n_sb = const.tile([32, 1], F32, tag="n_sb")
        nc.vector.tensor_copy(n_sb, ps_n[:, 0:1])
        inv_n = const.tile([32, 1], F32, tag="inv_n")
        nc.vector.reciprocal(inv_n, n_sb)
        ctx_sb = const.tile([32, H], F32, tag="ctx_sb")
        nc.scalar.activation(out=ctx_sb, in_=ps_ctx, func=AF.Copy, scale=inv_n[:, 0:1])
        ctxT = const.tile([128, 4, 32], F16, tag="ctxT")
        for hc in range(4):
            pst = pt.tile([128, 128], F32, tag="ptr")
            nc.tensor.transpose(out=pst[:, 0:32], in_=ctx_sb[:, hc * 128:(hc + 1) * 128],
                                identity=ident[0:32, 0:32])
            nc.vector.tensor_copy(ctxT[:, hc, :], pst[:, 0:32])

        # Bc[b, v] = ctx @ W2b : [32, VS] fp16
        bc16 = const.tile([32, VSP], F16, tag="bc16")
        for vi in range(NVC):
            wbt = outp.tile([128, 4, 512], F16, tag="w2t")
            nc.scalar.dma_start(out=wbt, in_=d["w2b"][vi])
            ps_bc = psm.tile([32, 512], F32, tag="psmall")
            for hc in range(4):
                nc.tensor.matmul(ps_bc, ctxT[:, hc, :], wbt[:, hc, :],
                                 start=(hc == 0), stop=(hc == 3))
            nc.vector.tensor_copy(bc16[:, vi * 512:(vi + 1) * 512], ps_bc)

        # ---------------- h0/c0 -> tile layout ----------------
        h_cur, c_cur = [], []
        for l in range(L):
            hs_in = stage.tile([32, H], F32, tag="ldf32")
            nc.sync.dma_start(out=hs_in, in_=d["h0"][l, :, :])
            cs_in = stage.tile([32, H], F32, tag="ldf32")
            nc.sync.dma_start(out=cs_in, in_=d["c0"][l, :, :])
            ht = chain.tile([128, 128], F16, tag=f"h{l}")
            ct = chain.tile([128, 128], F32, tag=f"c{l}")
            for hc in range(4):
                pst = pt.tile([128, 128], F32, tag="ptr")
                nc.tensor.transpose(out=pst[:, 0:32], in_=hs_in[:, hc * 128:(hc + 1) * 128],
                                    identity=ident[0:32, 0:32])
                nc.vector.tensor_copy(ht[:, hc * 32:(hc + 1) * 32], pst[:, 0:32])
                pst2 = pt.tile([128, 128], F32, tag="ptr")
                nc.tensor.transpose(out=pst2[:, 0:32], in_=cs_in[:, hc * 128:(hc + 1) * 128],
                                    identity=ident[0:32, 0:32])
                nc.vector.tensor_copy(ct[:, hc * 32:(hc + 1) * 32], pst2[:, 0:32])
            h_cur.append(ht)
            c_cur.append(ct)

        # ---------------- persistent big tiles ----------------
        h1T = big.tile([128, 4, TB], F16, tag="h1T")    # layer-1 outputs
        decT = big.tile([128, 4, TB], F16, tag="decT")  # layer-2 outputs

        def emit_xw_block(l, k, rhs_big):
            """xw (input projection + bias) for steps [16k, 16k+16) of
            layer l from rhs_big [128, hc, TB] fp16."""
            xwt = ring.tile([128, BLK, 512], F16, tag=f"xwr{l}")
            for gc in range(16):
                ps_x = pw.tile([128, 512], F32, tag="pwork")
                for hc in range(4):
                    nc.tensor.matmul(
                        ps_x, wih[l][:, hc, gc * 128:(gc + 1) * 128],
                        rhs_big[:, hc, k * 512:(k + 1) * 512],
                        start=(hc == 0), stop=(hc == 3))
                nc.scalar.activation(
                    out=xwt[:, :, gc * 32:(gc + 1) * 32],
                    in_=ps_x.rearrange("p (t b) -> p t b", b=32),
                    func=AF.Identity, bias=bias_sb[l][:, gc:gc + 1])
            return xwt

        def emit_step(l, t, xwt, final):
            """One LSTM step: 64 rec MMs + gate chain; updates h/c state."""
            ht_prev, ct_prev = h_cur[l], c_cur[l]
            ps_g = pg.tile([128, 512], F32, tag="pgate")
            nc.tensor.matmul(ps_g, ident16, xwt[:, t % BLK, :],
                             start=True, stop=False, skip_group_check=True)
            for gc in range(16):
                for hc in range(4):
                    nc.tensor.matmul(
                        ps_g[:, gc * 32:(gc + 1) * 32],
                        whh[l][:, hc, gc * 128:(gc + 1) * 128],
                        ht_prev[:, hc * 32:(hc + 1) * 32],
                        start=False, stop=(gc == 15 and hc == 3),
                        skip_group_check=True)
            sif = chain.tile([128, 384], F32, tag="sif")
            nc.scalar.activation(out=sif, in_=ps_g[:, 0:384], func=AF.Sigmoid)
            tg = chain.tile([128, 128], F32, tag="tg")
            nc.scalar.activation(out=tg, in_=ps_g[:, 384:512], func=AF.Tanh)
            t1 = chain.tile([128, 128], F32, tag="t1")
            nc.vector.tensor_mul(t1, sif[:, 0:128], tg)
            t2 = chain.tile([128, 128], F32, tag="t2")
            nc.vector.tensor_mul(t2, sif[:, 128:256], ct_prev)
            ct_new = chain.tile([128, 128], F32, tag=f"c{l}")
            nc.vector.tensor_add(ct_new, t1, t2)
            tcn = chain.tile([128, 128], F32, tag="tcn")
            nc.scalar.activation(out=tcn, in_=ct_new, func=AF.Tanh)
            ht_new = chain.tile([128, 128], F16, tag=f"h{l}")
            nc.vector.tensor_mul(ht_new, sif[:, 256:384], tcn)
            h_cur[l], c_cur[l] = ht_new, ct_new
            dst = h1T if l == 0 else decT
            for hc in range(4):
                nc.vector.tensor_copy(dst[:, hc, t * 32:(t + 1) * 32],
                                      ht_new[:, hc * 32:(hc + 1) * 32])
            if final:
                hf = const.tile([128, 128], F32, tag=f"hf{l}")
                nc.vector.tensor_mul(hf, sif[:, 256:384], tcn)
                nc.sync.dma_start(out=d["hso"][l],
                                  in_=hf.rearrange("p (j b) -> p j b", b=32))
                nc.sync.dma_start(out=d["cso"][l],
                                  in_=ct_new.rearrange("p (j b) -> p j b", b=32))

        logits_sched = []

        def emit_logits_group(tp, vi):
            """Logits for tb-chunk pair tp (tbc 2tp, 2tp+1), v-chunk vi."""
            w2t = outp.tile([128, 4, 512], F16, tag="w2t")
            nc.scalar.dma_start(out=w2t, in_=d["w2a"][vi])
            for tbc in (2 * tp, 2 * tp + 1):
                tb0 = tbc * 128
                ps_l = pw.tile([128, 512], F32, tag="pwork")
                for hc in range(4):
                    nc.tensor.matmul(
                        ps_l, decT[:, hc, tb0:tb0 + 128], w2t[:, hc, :],
                        start=(hc == 0), stop=False)
                nc.tensor.matmul(ps_l, base16T, bc16[:, vi * 512:(vi + 1) * 512],
                                 start=False, stop=True)
                ot = outp.tile([128, 512], F32, tag="ot")
                nc.vector.tensor_copy(ot, ps_l)
                nc.sync.dma_start(out=d["out"][tbc, vi], in_=ot)

        # ---------------- main interleaved schedule ----------------
        xw_cur = [None, None]
        xw_cur[0] = emit_xw_block(0, 0, xT)
        for slot in range(T + LAG):
            t1_ = slot
            t2_ = slot - LAG
            if t1_ < T:
                if t1_ % BLK == 0 and t1_ > 0:
                    xw_cur[0] = emit_xw_block(0, t1_ // BLK, xT)
                emit_step(0, t1_, xw_cur[0], final=(t1_ == T - 1))
            if logits_sched:
                emit_logits_group(*logits_sched.pop(0))
            if 0 <= t2_ < T:
                if t2_ % BLK == 0:
                    xw_cur[1] = emit_xw_block(1, t2_ // BLK, h1T)
                emit_step(1, t2_, xw_cur[1], final=(t2_ == T - 1))
                if t2_ % 8 == 7:
                    for vi in range(NVC):
                        logits_sched.append((t2_ // 8, vi))
            if logits_sched:
                emit_logits_group(*logits_sched.pop(0))
        while logits_sched:
            emit_logits_group(*logits_sched.pop(0))

    nc.finalize()
    return nc


_NC_CACHE = None


def kernel(**inputs):
    global _NC_CACHE
    tgt = np.asarray(inputs["tgt"]).astype(np.int32).reshape(TB, 1)
    emb = np.asarray(inputs["emb"], dtype=np.float32)
    enc = np.asarray(inputs["enc_output"], dtype=np.float32).reshape(S * B, H)
    msk = np.asarray(inputs["src_mask"]).astype(np.int32).reshape(S * B, 1)
    hid = np.asarray(inputs["hidden"], dtype=np.float32)
    cel = np.asarray(inputs["cell"], dtype=np.float32)
    w2 = np.asarray(inputs["W2"], dtype=np.float32)  # [V, 2H]

    # gate-permuted (i, f, o, g) fp16 weights, transposed to [H, 4H]
    perm = np.concatenate([np.arange(0, 1024), np.arange(1536, 2048),
                           np.arange(1024, 1536)])
    base = {"tgt": tgt, "emb": emb, "enc": enc, "msk": msk, "h0": hid, "c0": cel}
    for l in range(L):
        wihl = np.asarray(inputs[f"W_ih_l{l}"], dtype=np.float32)[perm]
        whhl = np.asarray(inputs[f"W_hh_l{l}"], dtype=np.float32)[perm]
        bl = (np.asarray(inputs[f"b_ih_l{l}"], dtype=np.float32)
              + np.asarray(inputs[f"b_hh_l{l}"], dtype=np.float32))[perm]
        # [H, 4H] -> [128, hc, 4H] (partition-contiguous)
        base[f"wih{l}"] = np.ascontiguousarray(
            wihl.T.reshape(4, 128, 4 * H).transpose(1, 0, 2)).astype(np.float16)
        base[f"whh{l}"] = np.ascontiguousarray(
            whhl.T.reshape(4, 128, 4 * H).transpose(1, 0, 2)).astype(np.float16)
        base[f"bias{l}"] = np.ascontiguousarray(bl.reshape(16, 128).T).astype(np.float32)

    w2a = w2[:, :H].T.astype(np.float16)  # [H, V]
    w2b = w2[:, H:].T.astype(np.float16)

    def shard_w2(w, k):
        # [H, VS] -> pad to [H, VSP] -> [vi, p, hc, vv] chunk-major
        ws = np.zeros((H, VSP), np.float16)
        ws[:, :VS] = w[:, k * VS:(k + 1) * VS]
        return np.ascontiguousarray(
            ws.reshape(4, 128, NVC, 512).transpose(2, 1, 0, 3))

    in_maps = []
    for k in range(NC):
        m = dict(base)
        m["w2a"] = shard_w2(w2a, k)
        m["w2b"] = shard_w2(w2b, k)
        in_maps.append(m)

    if _NC_CACHE is None:
        _NC_CACHE = build_nc()
    res = run_bass_kernel_spmd(_NC_CACHE, in_maps, list(range(NC)))
    outs = np.concatenate(
        [res.results[k]["out"].transpose(0, 2, 1, 3).reshape(TB, VSP)[:, :VS]
         .reshape(T, B, VS) for k in range(NC)], axis=-1)
    # device layout [l, p, j, b] -> [l, b, u=128j+p]
    hs = res.results[0]["hso"].transpose(0, 3, 2, 1).reshape(L, B, H)
    cs = res.results[0]["cso"].transpose(0, 3, 2, 1).reshape(L, B, H)
    return outs, hs, cs
